# revision 6
# baseline (speedup 1.0000x reference)
"""DGI (3-layer GraphSAGE encoder x2 + discriminator + pair MLP) on 8 trn2 cores.

Sharding: cores 0-3 compute the positive encode, cores 4-7 the negative
(feature-permutation folded into the layer-0 gather indices, so `x` is
never physically permuted).  Within each 4-core group every core owns a
contiguous quarter of each layer's destination nodes.  All cross-core
traffic is collectives: per-layer BN-stat AllReduce + AllGather of the
layer output (group-scoped), plus two tiny 8-core AllReduces for the
summary vector and the scalar loss.

Device compute is feature-major: a window of 128 dst nodes accumulates
meanT[f,d] in PSUM via one-hot matmuls over gathered edge tiles
(S[e,d] = (dst_local==iota)*recip_deg built in a single fused DVE op),
with the self term entering as an identity-one-hot pseudo tile.
"""

import math
import os
import sys

import numpy as np

_N = [100000, 50000, 25000, 12500]
_E_PAIR = 10000
_D = 128
_P = 128
_NCORES = 8
_NGRP = 4
_EPS = 1e-5
_PAIR_PER_CORE = None  # derived

_CONCOURSE_PATHS = ("/opt/trn_rl_repo", "/root/.axon_site/_ro/trn_rl_repo")


def _ensure_paths():
    for p in _CONCOURSE_PATHS:
        if os.path.isdir(p) and p not in sys.path:
            sys.path.insert(0, p)


def _cdiv(a, b):
    return -(-a // b)


def _col_major(vals, n_tiles, pad):
    """Place vals into a [128, n_tiles] array, element j -> [j%128, j//128]."""
    buf = np.full(n_tiles * _P, pad, dtype=vals.dtype)
    buf[: len(vals)] = vals
    return buf.reshape(n_tiles, _P).T.copy()


class _Meta:
    """Compile-time program structure (identical across the 8 cores)."""

    def __init__(self):
        self.tiles = []      # per layer: list of per-window tile counts (incl. self tile)
        self.n_win = []      # per layer: number of 128-dst windows
        self.T = []          # per layer: total tile columns
        self.q = []          # per layer: dst nodes per core
        self.n_pair_tiles = 0
        self.pair_per_core = 0

    def key(self):
        return (tuple(tuple(t) for t in self.tiles), self.n_pair_tiles)


def _prepare(inputs):
    """Host-side index preprocessing -> (in_maps list of 8 dicts, meta)."""
    x = np.asarray(inputs["x"], dtype=np.float32)
    perm = np.asarray(inputs["perm"], dtype=np.int64)
    srcs = [np.asarray(inputs[f"src{l}"], dtype=np.int64) for l in range(3)]
    dsts = [np.asarray(inputs[f"dst{l}"], dtype=np.int64) for l in range(3)]

    meta = _Meta()
    # per (layer, rank): sorted local edges
    layer_rank = []  # [l][r] -> dict
    for l in range(3):
        n_dst = _N[l + 1]
        q = n_dst // _NGRP
        deg = np.bincount(dsts[l], minlength=n_dst)
        recip = (1.0 / np.maximum(deg, 1)).astype(np.float32)
        n_win = _cdiv(q, _P)
        meta.n_win.append(n_win)
        meta.q.append(q)
        ranks = []
        for r in range(_NGRP):
            base = r * q
            sel = (dsts[l] >= base) & (dsts[l] < base + q)
            ds = dsts[l][sel] - base
            ss = srcs[l][sel]
            o = np.argsort(ds, kind="stable")
            ds = ds[o]
            ss = ss[o]
            bounds = np.searchsorted(ds, np.arange(n_win + 1) * _P)
            n_et = [
                _cdiv(int(bounds[w + 1] - bounds[w]), _P) for w in range(n_win)
            ]
            ranks.append(
                dict(base=base, ds=ds, ss=ss, bounds=bounds, n_et=n_et, recip=recip)
            )
        layer_rank.append(ranks)
        # aligned tile counts: 1 self tile + max edge tiles across ranks
        tiles = [1 + max(ranks[r]["n_et"][w] for r in range(_NGRP)) for w in range(n_win)]
        meta.tiles.append(tiles)
        meta.T.append(int(sum(tiles)))

    # pairs
    pair_a = np.concatenate(
        [np.asarray(inputs["pos_src"]), np.asarray(inputs["neg_src"])]
    ).astype(np.int64)
    pair_b = np.concatenate(
        [np.asarray(inputs["pos_dst"]), np.asarray(inputs["neg_dst"])]
    ).astype(np.int64)
    n_pairs = pair_a.shape[0]  # 2 * E_PAIR
    per_core = _cdiv(n_pairs, _NGRP)
    n_pt = _cdiv(per_core, _P)
    meta.n_pair_tiles = n_pt
    meta.pair_per_core = per_core

    # shared weight tensors (identical on every core)
    com = {}
    com["x"] = x
    for l in range(3):
        com[f"wself{l}"] = np.asarray(inputs[f"Wself{l}"], dtype=np.float32)
        com[f"wneigh{l}"] = np.asarray(inputs[f"Wneigh{l}"], dtype=np.float32)
        com[f"bias{l}"] = np.asarray(inputs[f"b{l}"], dtype=np.float32).reshape(_D, 1)
    for l in range(2):
        com[f"gamma{l}"] = np.asarray(inputs[f"gamma{l}"], dtype=np.float32).reshape(_D, 1)
        com[f"beta{l}"] = np.asarray(inputs[f"beta{l}"], dtype=np.float32).reshape(_D, 1)
    com["disc_wt"] = np.ascontiguousarray(
        np.asarray(inputs["disc_W"], dtype=np.float32).T
    )
    com["pw1"] = np.asarray(inputs["pW1"], dtype=np.float32)
    com["pw2"] = np.asarray(inputs["pW2"], dtype=np.float32)
    com["pw3"] = np.asarray(inputs["pW3"], dtype=np.float32).reshape(_D, 1)
    com["pb1"] = np.asarray(inputs["pb1"], dtype=np.float32).reshape(_D, 1)
    com["pb2"] = np.asarray(inputs["pb2"], dtype=np.float32).reshape(_D, 1)
    com["pb3"] = np.asarray(inputs["pb3"], dtype=np.float32).reshape(1, 1)
    com["iota"] = np.broadcast_to(
        np.arange(_P, dtype=np.float32)[None, :], (_P, _P)
    ).copy()
    # l2 window valid mask
    q2 = meta.q[2]
    m2 = np.zeros((_P, meta.n_win[2]), dtype=np.float32)
    for w in range(meta.n_win[2]):
        m2[: min(_P, q2 - w * _P), w] = 1.0
    com["mask_l2"] = m2

    in_maps = []
    for core in range(_NCORES):
        neg = core >= _NGRP
        r = core % _NGRP
        m = dict(com)
        m["mask_pos"] = np.full((_P, 1), 0.0 if neg else 1.0, dtype=np.float32)
        m["sigma"] = np.full((_P, 1), 1.0 if neg else -1.0, dtype=np.float32)
        for l in range(3):
            lr = layer_rank[l][r]
            q = meta.q[l]
            n_win = meta.n_win[l]
            T = meta.T[l]
            gidx = np.zeros((_P, T), dtype=np.int32)
            dl = np.full((_P, T), -1.0, dtype=np.float32)
            sc = np.zeros((_P, T), dtype=np.float32)
            t0 = 0
            for w in range(n_win):
                nt = meta.tiles[l][w]
                base_d = w * _P
                valid = min(_P, q - base_d)
                # self tile
                ids = lr["base"] + base_d + np.arange(valid, dtype=np.int64)
                if neg and l == 0:
                    ids = perm[ids]
                gidx[:valid, t0] = ids.astype(np.int32)
                dl[:valid, t0] = np.arange(valid, dtype=np.float32)
                sc[:valid, t0] = 1.0
                # edge tiles
                e0, e1 = int(lr["bounds"][w]), int(lr["bounds"][w + 1])
                ne = e1 - e0
                if ne > 0:
                    es = lr["ss"][e0:e1]
                    if neg and l == 0:
                        es = perm[es]
                    ed = (lr["ds"][e0:e1] - base_d).astype(np.float32)
                    esc = lr["recip"][lr["base"] + lr["ds"][e0:e1]]
                    n_ecols = nt - 1
                    gidx[:, t0 + 1 : t0 + nt] = _col_major(
                        es.astype(np.int32), n_ecols, 0
                    )
                    dl[:, t0 + 1 : t0 + nt] = _col_major(
                        ed, n_ecols, np.float32(-1.0)
                    )
                    sc[:, t0 + 1 : t0 + nt] = _col_major(
                        esc.astype(np.float32), n_ecols, np.float32(0.0)
                    )
                t0 += nt
            m[f"gidx{l}"] = gidx
            m[f"dlocal{l}"] = dl
            m[f"escale{l}"] = sc
        # pairs
        n_pt = meta.n_pair_tiles
        lo = r * meta.pair_per_core
        hi = min(lo + meta.pair_per_core, n_pairs)
        if neg:
            pa = np.zeros(0, dtype=np.int64)
            pb = np.zeros(0, dtype=np.int64)
        else:
            pa = pair_a[lo:hi]
            pb = pair_b[lo:hi]
        m["pair_a"] = _col_major(pa.astype(np.int32), n_pt, 0)
        m["pair_b"] = _col_major(pb.astype(np.int32), n_pt, 0)
        in_maps.append(m)
    return in_maps, meta


def _build(meta):
    """Build + compile the SPMD Bass program for the given meta."""
    _ensure_paths()
    import concourse.bass as bass
    import concourse.bacc as bacc
    import concourse.mybir as mybir
    import concourse.tile as tile
    from concourse.masks import make_identity

    f32 = mybir.dt.float32
    i32 = mybir.dt.int32
    OP = mybir.AluOpType
    AF = mybir.ActivationFunctionType
    AX = mybir.AxisListType

    rg_grp = [[0, 1, 2, 3], [4, 5, 6, 7]]
    rg_all = [list(range(8))]

    nc = bacc.Bacc(
        "TRN2", target_bir_lowering=False, debug=False, num_devices=_NCORES
    )

    # ---- I/O ----
    x = nc.dram_tensor("x", [_N[0], _D], f32, kind="ExternalInput")
    gidx = [
        nc.dram_tensor(f"gidx{l}", [_P, meta.T[l]], i32, kind="ExternalInput")
        for l in range(3)
    ]
    dlocal = [
        nc.dram_tensor(f"dlocal{l}", [_P, meta.T[l]], f32, kind="ExternalInput")
        for l in range(3)
    ]
    escale = [
        nc.dram_tensor(f"escale{l}", [_P, meta.T[l]], f32, kind="ExternalInput")
        for l in range(3)
    ]
    wself = [
        nc.dram_tensor(f"wself{l}", [_D, _D], f32, kind="ExternalInput")
        for l in range(3)
    ]
    wneigh = [
        nc.dram_tensor(f"wneigh{l}", [_D, _D], f32, kind="ExternalInput")
        for l in range(3)
    ]
    bias = [
        nc.dram_tensor(f"bias{l}", [_D, 1], f32, kind="ExternalInput")
        for l in range(3)
    ]
    gamma = [
        nc.dram_tensor(f"gamma{l}", [_D, 1], f32, kind="ExternalInput")
        for l in range(2)
    ]
    beta = [
        nc.dram_tensor(f"beta{l}", [_D, 1], f32, kind="ExternalInput")
        for l in range(2)
    ]
    disc_wt = nc.dram_tensor("disc_wt", [_D, _D], f32, kind="ExternalInput")
    pw1 = nc.dram_tensor("pw1", [_D, _D], f32, kind="ExternalInput")
    pw2 = nc.dram_tensor("pw2", [_D, _D], f32, kind="ExternalInput")
    pw3 = nc.dram_tensor("pw3", [_D, 1], f32, kind="ExternalInput")
    pb1 = nc.dram_tensor("pb1", [_D, 1], f32, kind="ExternalInput")
    pb2 = nc.dram_tensor("pb2", [_D, 1], f32, kind="ExternalInput")
    pb3 = nc.dram_tensor("pb3", [1, 1], f32, kind="ExternalInput")
    iota_d = nc.dram_tensor("iota", [_P, _P], f32, kind="ExternalInput")
    mask_l2_d = nc.dram_tensor(
        "mask_l2", [_P, meta.n_win[2]], f32, kind="ExternalInput"
    )
    mask_pos_d = nc.dram_tensor("mask_pos", [_P, 1], f32, kind="ExternalInput")
    sigma_d = nc.dram_tensor("sigma", [_P, 1], f32, kind="ExternalInput")
    pair_a_d = nc.dram_tensor(
        "pair_a", [_P, meta.n_pair_tiles], i32, kind="ExternalInput"
    )
    pair_b_d = nc.dram_tensor(
        "pair_b", [_P, meta.n_pair_tiles], i32, kind="ExternalInput"
    )

    pair_out = nc.dram_tensor(
        "pair_out", [meta.n_pair_tiles, _P], f32, kind="ExternalOutput"
    )
    loss_out = nc.dram_tensor("loss_out", [1, 1], f32, kind="ExternalOutput")

    # ---- internal DRAM ----
    h_loc = [nc.dram_tensor(f"hloc{l}", [meta.q[l], _D], f32) for l in range(3)]
    h_full = [
        nc.dram_tensor(f"hfull{l}", [_N[l + 1], _D], f32) for l in range(3)
    ]
    stats_in = [nc.dram_tensor(f"statin{l}", [_P, 2], f32) for l in range(2)]
    stats_out = [nc.dram_tensor(f"statout{l}", [_P, 2], f32) for l in range(2)]
    sumr_in = nc.dram_tensor("sumrin", [_P, 1], f32)
    sumr_out = nc.dram_tensor("sumrout", [_P, 1], f32, addr_space="Shared")
    loss_in = nc.dram_tensor("lossin", [1, 1], f32)
    loss_red = nc.dram_tensor("lossred", [1, 1], f32, addr_space="Shared")

    from contextlib import ExitStack

    with tile.TileContext(nc) as tc, ExitStack() as ctx:
        cst = ctx.enter_context(tc.tile_pool(name="cst", bufs=1))
        mp = ctx.enter_context(tc.tile_pool(name="mp", bufs=2))
        gp = ctx.enter_context(tc.tile_pool(name="gp", bufs=6))
        sp = ctx.enter_context(tc.tile_pool(name="sp", bufs=6))
        wp = ctx.enter_context(tc.tile_pool(name="wp", bufs=3))
        bp = ctx.enter_context(tc.tile_pool(name="bp", bufs=3))
        zp = ctx.enter_context(tc.tile_pool(name="zp", bufs=1))
        stp = ctx.enter_context(tc.tile_pool(name="stp", bufs=1))
        fp = ctx.enter_context(tc.tile_pool(name="fp", bufs=3))
        psA = ctx.enter_context(tc.tile_pool(name="psA", bufs=2, space="PSUM"))
        psB = ctx.enter_context(tc.tile_pool(name="psB", bufs=2, space="PSUM"))
        psZ = ctx.enter_context(tc.tile_pool(name="psZ", bufs=2, space="PSUM"))
        psT = ctx.enter_context(tc.tile_pool(name="psT", bufs=2, space="PSUM"))

        def _load_const(dram, shape, dtype=f32):
            t = cst.tile(shape, dtype, name=f"c_{dram.name}")
            nc.sync.dma_start(out=t[:], in_=dram[:, :])
            return t

        iota_t = _load_const(iota_d, [_P, _P])
        ident = cst.tile([_P, _P], f32, name="ident")
        make_identity(nc, ident[:])
        ws_t = [_load_const(wself[l], [_D, _D]) for l in range(3)]
        wn_t = [_load_const(wneigh[l], [_D, _D]) for l in range(3)]
        b_t = [_load_const(bias[l], [_D, 1]) for l in range(3)]
        g_t = [_load_const(gamma[l], [_D, 1]) for l in range(2)]
        be_t = [_load_const(beta[l], [_D, 1]) for l in range(2)]
        dwt_t = _load_const(disc_wt, [_D, _D])
        pw1_t = _load_const(pw1, [_D, _D])
        pw2_t = _load_const(pw2, [_D, _D])
        pw3_t = _load_const(pw3, [_D, 1])
        pb1_t = _load_const(pb1, [_D, 1])
        pb2_t = _load_const(pb2, [_D, 1])
        pb3_t = _load_const(pb3, [1, 1])
        ml2_t = _load_const(mask_l2_d, [_P, meta.n_win[2]])
        mpos_t = _load_const(mask_pos_d, [_P, 1])
        sig_t = _load_const(sigma_d, [_P, 1])

        zbufs = []
        for l in range(3):
            n_win = meta.n_win[l]
            q = meta.q[l]
            h_src = x if l == 0 else h_full[l - 1]
            zbuf = zp.tile([_P, n_win * _P], f32, name=f"zbuf{l}", tag=f"zbuf{l}")
            zbufs.append(zbuf)
            nc.gpsimd.memset(zbuf[:], 0.0)
            if l < 2:
                ssum = stp.tile([_P, n_win], f32, name=f"ssum{l}", tag=f"ssum{l}")
                ssq = stp.tile([_P, n_win], f32, name=f"ssq{l}", tag=f"ssq{l}")
            t_off = 0
            for w in range(n_win):
                nt = meta.tiles[l][w]
                valid = min(_P, q - w * _P)
                gi_w = mp.tile([_P, nt], i32, name="gi_w", tag="gi_w")
                nc.sync.dma_start(out=gi_w[:], in_=gidx[l][:, t_off : t_off + nt])
                dl_w = mp.tile([_P, nt], f32, name="dl_w", tag="dl_w")
                nc.sync.dma_start(out=dl_w[:], in_=dlocal[l][:, t_off : t_off + nt])
                sc_w = mp.tile([_P, nt], f32, name="sc_w", tag="sc_w")
                nc.sync.dma_start(out=sc_w[:], in_=escale[l][:, t_off : t_off + nt])
                agg_self = psA.tile([_P, _P], f32, name="agg_self", tag="psA")
                have_edges = nt > 1
                if have_edges:
                    agg_nei = psB.tile([_P, _P], f32, name="agg_nei", tag="psB")
                for t in range(nt):
                    G = gp.tile([_P, _P], f32, name="G", tag="G")
                    nc.gpsimd.indirect_dma_start(
                        out=G[:],
                        out_offset=None,
                        in_=h_src[:],
                        in_offset=bass.IndirectOffsetOnAxis(
                            ap=gi_w[:, t : t + 1], axis=0
                        ),
                    )
                    S = sp.tile([_P, _P], f32, name="S", tag="S")
                    nc.vector.tensor_scalar(
                        out=S[:],
                        in0=iota_t[:],
                        scalar1=dl_w[:, t : t + 1],
                        scalar2=sc_w[:, t : t + 1],
                        op0=OP.is_equal,
                        op1=OP.mult,
                    )
                    if t == 0:
                        nc.tensor.matmul(
                            out=agg_self[:], lhsT=G[:], rhs=S[:], start=True, stop=True
                        )
                    else:
                        nc.tensor.matmul(
                            out=agg_nei[:],
                            lhsT=G[:],
                            rhs=S[:],
                            start=(t == 1),
                            stop=(t == nt - 1),
                        )
                a_self = wp.tile([_P, _P], f32, name="a_self", tag="a_self")
                nc.vector.tensor_copy(out=a_self[:], in_=agg_self[:])
                if have_edges:
                    a_nei = wp.tile([_P, _P], f32, name="a_nei", tag="a_nei")
                    nc.vector.tensor_copy(out=a_nei[:], in_=agg_nei[:])
                zt = psZ.tile([_P, _P], f32, name="zt", tag="psZ")
                nc.tensor.matmul(
                    out=zt[:],
                    lhsT=ws_t[l][:],
                    rhs=a_self[:],
                    start=True,
                    stop=not have_edges,
                )
                if have_edges:
                    nc.tensor.matmul(
                        out=zt[:], lhsT=wn_t[l][:], rhs=a_nei[:], start=False, stop=True
                    )
                dstcol = zbufs[l][:, w * _P : w * _P + valid]
                if l < 2:
                    nc.scalar.activation(
                        out=dstcol,
                        in_=zt[:, :valid],
                        func=AF.Identity,
                        bias=b_t[l][:, :1],
                        accum_out=ssum[:, w : w + 1],
                    )
                    sqs = bp.tile([_P, _P], f32, name="sqs", tag="sqs")
                    nc.scalar.activation(
                        out=sqs[:, :valid],
                        in_=zt[:, :valid],
                        func=AF.Square,
                        bias=b_t[l][:, :1],
                        accum_out=ssq[:, w : w + 1],
                    )
                else:
                    nc.scalar.activation(
                        out=dstcol,
                        in_=zt[:, :valid],
                        func=AF.Identity,
                        bias=b_t[l][:, :1],
                    )
                t_off += nt

            if l < 2:
                # group all-reduce of BN stats
                ssum_tot = stp.tile([_P, 1], f32, name=f"ssumtot{l}", tag=f"sst{l}")
                nc.vector.tensor_reduce(
                    out=ssum_tot[:], in_=ssum[:], axis=AX.X, op=OP.add
                )
                ssq_tot = stp.tile([_P, 1], f32, name=f"ssqtot{l}", tag=f"ssqt{l}")
                nc.vector.tensor_reduce(
                    out=ssq_tot[:], in_=ssq[:], axis=AX.X, op=OP.add
                )
                nc.sync.dma_start(out=stats_in[l][:, 0:1], in_=ssum_tot[:])
                nc.sync.dma_start(out=stats_in[l][:, 1:2], in_=ssq_tot[:])
                nc.gpsimd.collective_compute(
                    "AllReduce",
                    OP.add,
                    replica_groups=rg_grp,
                    ins=[stats_in[l].ap()],
                    outs=[stats_out[l].ap()],
                )
                st = stp.tile([_P, 2], f32, name=f"st{l}", tag=f"stld{l}")
                nc.sync.dma_start(out=st[:], in_=stats_out[l][:, :])
                inv_n = 1.0 / float(_N[l + 1])
                mean = stp.tile([_P, 1], f32, name=f"mean{l}", tag=f"mean{l}")
                nc.scalar.activation(
                    out=mean[:], in_=st[:, 0:1], func=AF.Copy, scale=inv_n
                )
                ex2 = stp.tile([_P, 1], f32, name=f"ex2{l}", tag=f"ex2{l}")
                nc.scalar.activation(
                    out=ex2[:], in_=st[:, 1:2], func=AF.Copy, scale=inv_n
                )
                m2 = stp.tile([_P, 1], f32, name=f"m2{l}", tag=f"m2{l}")
                nc.vector.tensor_tensor(
                    out=m2[:], in0=mean[:], in1=mean[:], op=OP.mult
                )
                var = stp.tile([_P, 1], f32, name=f"var{l}", tag=f"var{l}")
                nc.vector.tensor_tensor(
                    out=var[:], in0=ex2[:], in1=m2[:], op=OP.subtract
                )
                vpe = stp.tile([_P, 1], f32, name=f"vpe{l}", tag=f"vpe{l}")
                nc.vector.tensor_scalar_add(out=vpe[:], in0=var[:], scalar1=float(_EPS))
                rv = stp.tile([_P, 1], f32, name=f"rv{l}", tag=f"rv{l}")
                nc.vector.reciprocal(out=rv[:], in_=vpe[:])
                rstd = stp.tile([_P, 1], f32, name=f"rstd{l}", tag=f"rstd{l}")
                nc.scalar.activation(out=rstd[:], in_=rv[:], func=AF.Sqrt)
                scl = stp.tile([_P, 1], f32, name=f"scl{l}", tag=f"scl{l}")
                nc.vector.tensor_tensor(
                    out=scl[:], in0=g_t[l][:], in1=rstd[:], op=OP.mult
                )
                msc = stp.tile([_P, 1], f32, name=f"msc{l}", tag=f"msc{l}")
                nc.vector.tensor_tensor(
                    out=msc[:], in0=mean[:], in1=scl[:], op=OP.mult
                )
                shift = stp.tile([_P, 1], f32, name=f"shift{l}", tag=f"shift{l}")
                nc.vector.tensor_tensor(
                    out=shift[:], in0=be_t[l][:], in1=msc[:], op=OP.subtract
                )

            # pass B: (BN+ReLU) -> transpose -> node-major local shard
            for w in range(n_win):
                valid = min(_P, q - w * _P)
                src_cols = zbufs[l][:, w * _P : (w + 1) * _P]
                if l < 2:
                    hb = bp.tile([_P, _P], f32, name="hb", tag="hb")
                    nc.scalar.activation(
                        out=hb[:],
                        in_=src_cols,
                        func=AF.Relu,
                        scale=scl[:, :1],
                        bias=shift[:, :1],
                    )
                    tsrc = hb
                else:
                    tsrc = None
                tp = psT.tile([_P, _P], f32, name="tp", tag="psT")
                nc.tensor.transpose(
                    out=tp[:],
                    in_=(tsrc[:] if tsrc is not None else src_cols),
                    identity=ident[:],
                )
                hn = bp.tile([_P, _P], f32, name="hn", tag="hn")
                nc.vector.tensor_copy(out=hn[:valid, :], in_=tp[:valid, :])
                nc.sync.dma_start(
                    out=h_loc[l][w * _P : w * _P + valid, :], in_=hn[:valid, :]
                )
            nc.gpsimd.collective_compute(
                "AllGather",
                OP.bypass,
                replica_groups=rg_grp,
                ins=[h_loc[l].ap()],
                outs=[h_full[l].ap()],
            )

        # ---- final stage ----
        # summary partial (masked) over posT = zbufs[2]
        spart = fp.tile([_P, 1], f32, name="spart", tag="spart")
        nc.vector.tensor_reduce(out=spart[:], in_=zbufs[2][:], axis=AX.X, op=OP.add)
        nc.vector.tensor_scalar_mul(out=spart[:], in0=spart[:], scalar1=mpos_t[:, :1])
        nc.sync.dma_start(out=sumr_in[:, :], in_=spart[:])
        nc.gpsimd.collective_compute(
            "AllReduce",
            OP.add,
            replica_groups=rg_all,
            ins=[sumr_in.ap()],
            outs=[sumr_out.ap()],
        )
        ssumv = fp.tile([_P, 1], f32, name="ssumv", tag="ssumv")
        nc.sync.dma_start(out=ssumv[:], in_=sumr_out[:, :])
        summary = fp.tile([_P, 1], f32, name="summary", tag="summary")
        nc.scalar.activation(
            out=summary[:], in_=ssumv[:], func=AF.Sigmoid, scale=1.0 / float(_N[3])
        )
        ws_ps = psA.tile([_P, 1], f32, name="ws_ps", tag="psA")
        nc.tensor.matmul(
            out=ws_ps[:], lhsT=dwt_t[:], rhs=summary[:], start=True, stop=True
        )
        wsv = fp.tile([_P, 1], f32, name="wsv", tag="wsv")
        nc.vector.tensor_copy(out=wsv[:], in_=ws_ps[:])

        loss_ps = psT.tile([1, 1], f32, name="loss_ps", tag="psT")
        for w in range(meta.n_win[2]):
            sc_ps = psB.tile([_P, 1], f32, name="sc_ps", tag="psB")
            nc.tensor.matmul(
                out=sc_ps[:],
                lhsT=zbufs[2][:, w * _P : (w + 1) * _P],
                rhs=wsv[:],
                start=True,
                stop=True,
            )
            spu = fp.tile([_P, 1], f32, name="spu", tag="spu")
            nc.vector.tensor_scalar(
                out=spu[:],
                in0=sc_ps[:],
                scalar1=sig_t[:, :1],
                scalar2=30.0,
                op0=OP.mult,
                op1=OP.min,
            )
            spe = fp.tile([_P, 1], f32, name="spe", tag="spe")
            nc.scalar.activation(out=spe[:], in_=spu[:], func=AF.Exp)
            sp1 = fp.tile([_P, 1], f32, name="sp1", tag="sp1")
            nc.vector.tensor_scalar_add(out=sp1[:], in0=spe[:], scalar1=1.0)
            spl = fp.tile([_P, 1], f32, name="spl", tag="spl")
            nc.scalar.activation(out=spl[:], in_=sp1[:], func=AF.Ln)
            nc.tensor.matmul(
                out=loss_ps[:],
                lhsT=spl[:],
                rhs=ml2_t[:, w : w + 1],
                start=(w == 0),
                stop=(w == meta.n_win[2] - 1),
            )
        lsum = fp.tile([1, 1], f32, name="lsum", tag="lsum")
        nc.scalar.activation(
            out=lsum[:], in_=loss_ps[:], func=AF.Copy, scale=1.0 / float(_N[3])
        )
        nc.sync.dma_start(out=loss_in[:, :], in_=lsum[:])
        nc.gpsimd.collective_compute(
            "AllReduce",
            OP.add,
            replica_groups=rg_all,
            ins=[loss_in.ap()],
            outs=[loss_red.ap()],
        )
        lfin = fp.tile([1, 1], f32, name="lfin", tag="lfin")
        nc.sync.dma_start(out=lfin[:], in_=loss_red[:, :])
        nc.sync.dma_start(out=loss_out[:, :], in_=lfin[:])

        # predictor MLP over pair tiles (positive_full = h_full[2])
        pa_t = cst.tile([_P, meta.n_pair_tiles], i32, name="pa_t")
        nc.sync.dma_start(out=pa_t[:], in_=pair_a_d[:, :])
        pb_t = cst.tile([_P, meta.n_pair_tiles], i32, name="pb_t")
        nc.sync.dma_start(out=pb_t[:], in_=pair_b_d[:, :])
        for t in range(meta.n_pair_tiles):
            A = gp.tile([_P, _P], f32, name="A", tag="G")
            nc.gpsimd.indirect_dma_start(
                out=A[:],
                out_offset=None,
                in_=h_full[2][:],
                in_offset=bass.IndirectOffsetOnAxis(ap=pa_t[:, t : t + 1], axis=0),
            )
            B = sp.tile([_P, _P], f32, name="B", tag="S")
            nc.gpsimd.indirect_dma_start(
                out=B[:],
                out_offset=None,
                in_=h_full[2][:],
                in_offset=bass.IndirectOffsetOnAxis(ap=pb_t[:, t : t + 1], axis=0),
            )
            Z = bp.tile([_P, _P], f32, name="Z", tag="hb")
            nc.vector.tensor_tensor(out=Z[:], in0=A[:], in1=B[:], op=OP.mult)
            ztp = psT.tile([_P, _P], f32, name="ztp", tag="psT")
            nc.tensor.transpose(out=ztp[:], in_=Z[:], identity=ident[:])
            zin = bp.tile([_P, _P], f32, name="zin", tag="hn")
            nc.vector.tensor_copy(out=zin[:], in_=ztp[:])
            z1p = psA.tile([_P, _P], f32, name="z1p", tag="psA")
            nc.tensor.matmul(
                out=z1p[:], lhsT=pw1_t[:], rhs=zin[:], start=True, stop=True
            )
            z1 = wp.tile([_P, _P], f32, name="z1", tag="a_self")
            nc.scalar.activation(
                out=z1[:], in_=z1p[:], func=AF.Relu, bias=pb1_t[:, :1]
            )
            z2p = psB.tile([_P, _P], f32, name="z2p", tag="psB")
            nc.tensor.matmul(
                out=z2p[:], lhsT=pw2_t[:], rhs=z1[:], start=True, stop=True
            )
            z2 = wp.tile([_P, _P], f32, name="z2", tag="a_nei")
            nc.scalar.activation(
                out=z2[:], in_=z2p[:], func=AF.Relu, bias=pb2_t[:, :1]
            )
            hp_ps = psZ.tile([1, _P], f32, name="hp_ps", tag="psZ")
            nc.tensor.matmul(
                out=hp_ps[:], lhsT=pw3_t[:], rhs=z2[:], start=True, stop=True
            )
            ho = fp.tile([1, _P], f32, name="ho", tag="ho")
            nc.scalar.activation(
                out=ho[:], in_=hp_ps[:], func=AF.Identity, bias=pb3_t[:1, :1]
            )
            nc.sync.dma_start(out=pair_out[t : t + 1, :], in_=ho[:])

    nc.compile()
    return nc


_CACHE = {}


def _get_program(meta):
    k = meta.key()
    if k not in _CACHE:
        _CACHE[k] = _build(meta)
    return _CACHE[k]


LAST_RESULTS = None


def _install_ntff_hook():
    """Provide antenv.axon_hooks (missing in this image) so
    run_bass_kernel_spmd(trace=True) can capture NTFF profiles."""
    import types
    import ctypes
    import contextlib

    try:
        from antenv.axon_hooks import get_axon_ntff_profile_hook  # noqa: F401

        return True
    except ImportError:
        pass
    so_path = "/opt/axon/libaxon_pjrt.so"
    if not os.path.exists(so_path):
        return False
    lib = ctypes.CDLL(so_path)
    if not hasattr(lib, "axon_start_nrt_profile"):
        return False
    lib.axon_start_nrt_profile.argtypes = [
        ctypes.POINTER(ctypes.c_int64),
        ctypes.c_size_t,
    ]
    lib.axon_start_nrt_profile.restype = ctypes.c_int64
    lib.axon_stop_nrt_profile.argtypes = [ctypes.c_char_p]
    lib.axon_stop_nrt_profile.restype = ctypes.c_int64

    @contextlib.contextmanager
    def _hook(output_dir, device_ids):
        import jax

        jax.devices()
        if device_ids:
            ids = (ctypes.c_int64 * len(device_ids))(*device_ids)
            rc = lib.axon_start_nrt_profile(ids, len(device_ids))
        else:
            rc = lib.axon_start_nrt_profile(None, 0)
        if rc != 0:
            raise RuntimeError(f"axon_start_nrt_profile rc={rc}")
        try:
            yield
        finally:
            n = lib.axon_stop_nrt_profile(str(output_dir).encode())
            print(f"ntff profile: {n} file(s) -> {output_dir}", file=sys.stderr)

    mod = types.ModuleType("antenv.axon_hooks")
    mod.get_axon_ntff_profile_hook = lambda: _hook
    mod.set_axon_ntff_profile_hook = lambda h: None
    import antenv

    antenv.axon_hooks = mod
    sys.modules["antenv.axon_hooks"] = mod
    return True


def kernel(**inputs):
    global LAST_RESULTS
    _ensure_paths()
    from concourse import bass_utils

    in_maps, meta = _prepare(inputs)
    nc = _get_program(meta)
    trace = bool(os.environ.get("KERNEL_TRACE"))
    if trace:
        trace = _install_ntff_hook()
    res = bass_utils.run_bass_kernel_spmd(
        nc, in_maps, core_ids=list(range(_NCORES)), trace=trace
    )
    LAST_RESULTS = res

    n_pairs = 2 * _E_PAIR
    flat = np.concatenate(
        [res.results[c]["pair_out"].reshape(-1)[: meta.pair_per_core] for c in range(_NGRP)]
    )[:n_pairs]
    h_pos = flat[:_E_PAIR].reshape(_E_PAIR, 1).astype(np.float32)
    h_neg = flat[_E_PAIR:].reshape(_E_PAIR, 1).astype(np.float32)
    loss = np.float32(res.results[0]["loss_out"].reshape(-1)[0])
    return h_pos, h_neg, loss


# revision 10
# speedup vs baseline: 1.0584x; 1.0584x over previous
"""DGI (3-layer GraphSAGE encoder x2 + discriminator + pair MLP) on 8 trn2 cores.

Sharding: cores 0-3 compute the positive encode, cores 4-7 the negative
(feature-permutation folded into the layer-0 gather indices, so `x` is
never physically permuted).  Within each 4-core group every core owns a
contiguous quarter of each layer's destination nodes.  All cross-core
traffic is collectives: per-layer BN-stat AllReduce + AllGather of the
layer output (group-scoped), plus two tiny 8-core AllReduces for the
summary vector and the scalar loss.

Device compute is feature-major: a window of 128 dst nodes accumulates
meanT[f,d] in PSUM via one-hot matmuls over gathered edge tiles
(S[e,d] = (dst_local==iota)*recip_deg built in a single fused DVE op),
with the self term entering as an identity-one-hot pseudo tile.  The
edge pipeline (h tables, gathers, one-hots, aggregation matmuls) runs
in bf16 with fp32 PSUM accumulation; BN statistics, batch-norm, scores
and the loss are fp32.
"""

import os
import sys

import numpy as np

_N = [100000, 50000, 25000, 12500]
_E_PAIR = 10000
_D = 128
_P = 128
_NCORES = 8
_NGRP = 4
_EPS = 1e-5
_GK = 1  # gather batch: index columns per indirect-DMA instruction

_CONCOURSE_PATHS = ("/opt/trn_rl_repo", "/root/.axon_site/_ro/trn_rl_repo")


def _ensure_paths():
    for p in _CONCOURSE_PATHS:
        if os.path.isdir(p) and p not in sys.path:
            sys.path.insert(0, p)


def _bf16_np():
    import ml_dtypes

    return np.dtype(ml_dtypes.bfloat16)


def _cdiv(a, b):
    return -(-a // b)


def _col_major(vals, n_tiles, pad, dtype):
    """Place vals into a [128, n_tiles] array, element j -> [j%128, j//128]."""
    buf = np.full(n_tiles * _P, pad, dtype=dtype)
    buf[: len(vals)] = vals.astype(dtype)
    return buf.reshape(n_tiles, _P).T.copy()


class _Meta:
    """Compile-time program structure (identical across the 8 cores)."""

    def __init__(self):
        self.tiles = []      # per layer: per-window tile counts (incl. self tile)
        self.n_win = []      # per layer: number of 128-dst windows
        self.T = []          # per layer: total tile columns
        self.q = []          # per layer: dst nodes per core
        self.n_pair_tiles = 0
        self.pair_per_core = 0

    def key(self):
        return (tuple(tuple(t) for t in self.tiles), self.n_pair_tiles, _GK)


def _prepare(inputs):
    """Host-side index preprocessing -> (in_maps list of 8 dicts, meta)."""
    bf16 = _bf16_np()
    x = np.asarray(inputs["x"], dtype=np.float32)
    perm = np.asarray(inputs["perm"], dtype=np.int64)
    srcs = [np.asarray(inputs[f"src{l}"], dtype=np.int64) for l in range(3)]
    dsts = [np.asarray(inputs[f"dst{l}"], dtype=np.int64) for l in range(3)]

    meta = _Meta()
    layer_rank = []  # [l][r] -> dict
    for l in range(3):
        n_dst = _N[l + 1]
        q = n_dst // _NGRP
        deg = np.bincount(dsts[l], minlength=n_dst)
        recip = (1.0 / np.maximum(deg, 1)).astype(np.float32)
        n_win = _cdiv(q, _P)
        meta.n_win.append(n_win)
        meta.q.append(q)
        ranks = []
        for r in range(_NGRP):
            base = r * q
            sel = (dsts[l] >= base) & (dsts[l] < base + q)
            ds = dsts[l][sel] - base
            ss = srcs[l][sel]
            o = np.argsort(ds, kind="stable")
            ds = ds[o]
            ss = ss[o]
            bounds = np.searchsorted(ds, np.arange(n_win + 1) * _P)
            n_et = [_cdiv(int(bounds[w + 1] - bounds[w]), _P) for w in range(n_win)]
            ranks.append(
                dict(base=base, ds=ds, ss=ss, bounds=bounds, n_et=n_et, recip=recip)
            )
        layer_rank.append(ranks)
        tiles = [
            1 + max(ranks[r]["n_et"][w] for r in range(_NGRP)) for w in range(n_win)
        ]
        meta.tiles.append(tiles)
        meta.T.append(int(sum(tiles)))

    pair_a = np.concatenate(
        [np.asarray(inputs["pos_src"]), np.asarray(inputs["neg_src"])]
    ).astype(np.int64)
    pair_b = np.concatenate(
        [np.asarray(inputs["pos_dst"]), np.asarray(inputs["neg_dst"])]
    ).astype(np.int64)
    n_pairs = pair_a.shape[0]
    per_core = _cdiv(n_pairs, _NGRP)
    meta.n_pair_tiles = _cdiv(per_core, _P)
    meta.pair_per_core = per_core

    com = {}
    com["xb"] = x.astype(bf16)
    for l in range(3):
        com[f"wself{l}"] = np.asarray(inputs[f"Wself{l}"], dtype=np.float32).astype(bf16)
        com[f"wneigh{l}"] = np.asarray(inputs[f"Wneigh{l}"], dtype=np.float32).astype(bf16)
        com[f"bias{l}"] = np.asarray(inputs[f"b{l}"], dtype=np.float32).reshape(_D, 1)
    for l in range(2):
        com[f"gamma{l}"] = np.asarray(inputs[f"gamma{l}"], dtype=np.float32).reshape(_D, 1)
        com[f"beta{l}"] = np.asarray(inputs[f"beta{l}"], dtype=np.float32).reshape(_D, 1)
    com["disc_wt"] = np.ascontiguousarray(
        np.asarray(inputs["disc_W"], dtype=np.float32).T
    )
    com["pw1"] = np.asarray(inputs["pW1"], dtype=np.float32).astype(bf16)
    com["pw2"] = np.asarray(inputs["pW2"], dtype=np.float32).astype(bf16)
    com["pw3"] = np.asarray(inputs["pW3"], dtype=np.float32).reshape(_D, 1).astype(bf16)
    com["pb1"] = np.asarray(inputs["pb1"], dtype=np.float32).reshape(_D, 1)
    com["pb2"] = np.asarray(inputs["pb2"], dtype=np.float32).reshape(_D, 1)
    com["pb3"] = np.asarray(inputs["pb3"], dtype=np.float32).reshape(1, 1)
    com["iota"] = np.broadcast_to(
        np.arange(_P, dtype=np.float32)[None, :], (_P, _P)
    ).astype(bf16)
    com["ident_b"] = np.eye(_P, dtype=np.float32).astype(bf16)
    q2 = meta.q[2]
    m2 = np.zeros((_P, meta.n_win[2]), dtype=np.float32)
    for w in range(meta.n_win[2]):
        m2[: min(_P, q2 - w * _P), w] = 1.0
    com["mask_l2"] = m2

    in_maps = []
    for core in range(_NCORES):
        neg = core >= _NGRP
        r = core % _NGRP
        m = dict(com)
        m["mask_pos"] = np.full((_P, 1), 0.0 if neg else 1.0, dtype=np.float32)
        m["sigma"] = np.full((_P, 1), 1.0 if neg else -1.0, dtype=np.float32)
        for l in range(3):
            lr = layer_rank[l][r]
            q = meta.q[l]
            n_win = meta.n_win[l]
            T = meta.T[l]
            gidx = np.zeros((_P, T), dtype=np.int32)
            dl = np.full((_P, T), -1.0, dtype=np.float32)
            sc = np.zeros((_P, T), dtype=np.float32)
            t0 = 0
            for w in range(n_win):
                nt = meta.tiles[l][w]
                base_d = w * _P
                valid = min(_P, q - base_d)
                ids = lr["base"] + base_d + np.arange(valid, dtype=np.int64)
                if neg and l == 0:
                    ids = perm[ids]
                gidx[:valid, t0] = ids.astype(np.int32)
                dl[:valid, t0] = np.arange(valid, dtype=np.float32)
                sc[:valid, t0] = np.float32(1.0)
                e0, e1 = int(lr["bounds"][w]), int(lr["bounds"][w + 1])
                ne = e1 - e0
                if ne > 0:
                    es = lr["ss"][e0:e1]
                    if neg and l == 0:
                        es = perm[es]
                    ed = (lr["ds"][e0:e1] - base_d).astype(np.float32)
                    esc = lr["recip"][lr["base"] + lr["ds"][e0:e1]]
                    n_ecols = nt - 1
                    gidx[:, t0 + 1 : t0 + nt] = _col_major(
                        es, n_ecols, 0, np.int32
                    )
                    dl[:, t0 + 1 : t0 + nt] = _col_major(ed, n_ecols, -1.0, np.float32)
                    sc[:, t0 + 1 : t0 + nt] = _col_major(esc, n_ecols, 0.0, np.float32)
                t0 += nt
            m[f"gidx{l}"] = gidx
            m[f"dlocal{l}"] = dl
            m[f"escale{l}"] = sc
        n_pt = meta.n_pair_tiles
        lo = r * meta.pair_per_core
        hi = min(lo + meta.pair_per_core, n_pairs)
        if neg:
            pa = np.zeros(0, dtype=np.int64)
            pb = np.zeros(0, dtype=np.int64)
        else:
            pa = pair_a[lo:hi]
            pb = pair_b[lo:hi]
        m["pair_a"] = _col_major(pa, n_pt, 0, np.int32)
        m["pair_b"] = _col_major(pb, n_pt, 0, np.int32)
        in_maps.append(m)
    return in_maps, meta


def _build(meta):
    """Build + compile the SPMD Bass program for the given meta."""
    _ensure_paths()
    import concourse.bass as bass
    import concourse.bacc as bacc
    import concourse.mybir as mybir
    import concourse.tile as tile
    from concourse.masks import make_identity
    from contextlib import ExitStack

    f32 = mybir.dt.float32
    bf16 = mybir.dt.bfloat16
    i32 = mybir.dt.int32
    OP = mybir.AluOpType
    AF = mybir.ActivationFunctionType
    AX = mybir.AxisListType

    rg_grp = [[0, 1, 2, 3], [4, 5, 6, 7]]
    rg_all = [list(range(8))]

    nc = bacc.Bacc(
        "TRN2", target_bir_lowering=False, debug=False, num_devices=_NCORES
    )

    # ---- I/O ----
    xb = nc.dram_tensor("xb", [_N[0], _D], bf16, kind="ExternalInput")
    gidx = [
        nc.dram_tensor(f"gidx{l}", [_P, meta.T[l]], i32, kind="ExternalInput")
        for l in range(3)
    ]
    dlocal = [
        nc.dram_tensor(f"dlocal{l}", [_P, meta.T[l]], f32, kind="ExternalInput")
        for l in range(3)
    ]
    escale = [
        nc.dram_tensor(f"escale{l}", [_P, meta.T[l]], f32, kind="ExternalInput")
        for l in range(3)
    ]
    wself = [
        nc.dram_tensor(f"wself{l}", [_D, _D], bf16, kind="ExternalInput")
        for l in range(3)
    ]
    wneigh = [
        nc.dram_tensor(f"wneigh{l}", [_D, _D], bf16, kind="ExternalInput")
        for l in range(3)
    ]
    bias = [
        nc.dram_tensor(f"bias{l}", [_D, 1], f32, kind="ExternalInput")
        for l in range(3)
    ]
    gamma = [
        nc.dram_tensor(f"gamma{l}", [_D, 1], f32, kind="ExternalInput")
        for l in range(2)
    ]
    beta = [
        nc.dram_tensor(f"beta{l}", [_D, 1], f32, kind="ExternalInput")
        for l in range(2)
    ]
    disc_wt = nc.dram_tensor("disc_wt", [_D, _D], f32, kind="ExternalInput")
    pw1 = nc.dram_tensor("pw1", [_D, _D], bf16, kind="ExternalInput")
    pw2 = nc.dram_tensor("pw2", [_D, _D], bf16, kind="ExternalInput")
    pw3 = nc.dram_tensor("pw3", [_D, 1], bf16, kind="ExternalInput")
    pb1 = nc.dram_tensor("pb1", [_D, 1], f32, kind="ExternalInput")
    pb2 = nc.dram_tensor("pb2", [_D, 1], f32, kind="ExternalInput")
    pb3 = nc.dram_tensor("pb3", [1, 1], f32, kind="ExternalInput")
    iota_d = nc.dram_tensor("iota", [_P, _P], bf16, kind="ExternalInput")
    ident_b_d = nc.dram_tensor("ident_b", [_P, _P], bf16, kind="ExternalInput")
    mask_l2_d = nc.dram_tensor(
        "mask_l2", [_P, meta.n_win[2]], f32, kind="ExternalInput"
    )
    mask_pos_d = nc.dram_tensor("mask_pos", [_P, 1], f32, kind="ExternalInput")
    sigma_d = nc.dram_tensor("sigma", [_P, 1], f32, kind="ExternalInput")
    pair_a_d = nc.dram_tensor(
        "pair_a", [_P, meta.n_pair_tiles], i32, kind="ExternalInput"
    )
    pair_b_d = nc.dram_tensor(
        "pair_b", [_P, meta.n_pair_tiles], i32, kind="ExternalInput"
    )

    pair_out = nc.dram_tensor(
        "pair_out", [meta.n_pair_tiles, _P], f32, kind="ExternalOutput"
    )
    loss_out = nc.dram_tensor("loss_out", [1, 1], f32, kind="ExternalOutput")

    # ---- internal DRAM ----
    h_loc = [
        nc.dram_tensor(f"hloc{l}", [meta.q[l], _D], bf16 if l < 2 else f32)
        for l in range(3)
    ]
    h_full = [
        nc.dram_tensor(f"hfull{l}", [_N[l + 1], _D], bf16 if l < 2 else f32)
        for l in range(3)
    ]
    stats_in = [nc.dram_tensor(f"statin{l}", [_P, 2], f32) for l in range(2)]
    stats_out = [nc.dram_tensor(f"statout{l}", [_P, 2], f32) for l in range(2)]
    sumr_in = nc.dram_tensor("sumrin", [_P, 1], f32)
    sumr_out = nc.dram_tensor("sumrout", [_P, 1], f32, addr_space="Shared")
    loss_in = nc.dram_tensor("lossin", [1, 1], f32)
    loss_red = nc.dram_tensor("lossred", [1, 1], f32, addr_space="Shared")

    with tile.TileContext(nc) as tc, ExitStack() as ctx:
        cst = ctx.enter_context(tc.tile_pool(name="cst", bufs=1))
        mlp = ctx.enter_context(tc.tile_pool(name="mlp", bufs=1))
        gp = ctx.enter_context(tc.tile_pool(name="gp", bufs=6))
        sp = ctx.enter_context(tc.tile_pool(name="sp", bufs=6))
        wp = ctx.enter_context(tc.tile_pool(name="wp", bufs=3))
        bp = ctx.enter_context(tc.tile_pool(name="bp", bufs=3))
        zp = ctx.enter_context(tc.tile_pool(name="zp", bufs=1))
        stp = ctx.enter_context(tc.tile_pool(name="stp", bufs=1))
        fp = ctx.enter_context(tc.tile_pool(name="fp", bufs=3))
        psA = ctx.enter_context(tc.tile_pool(name="psA", bufs=2, space="PSUM"))
        psB = ctx.enter_context(tc.tile_pool(name="psB", bufs=2, space="PSUM"))
        psZ = ctx.enter_context(tc.tile_pool(name="psZ", bufs=2, space="PSUM"))
        psT = ctx.enter_context(tc.tile_pool(name="psT", bufs=2, space="PSUM"))

        def _load_const(dram, shape, dtype=f32):
            t = cst.tile(shape, dtype, name=f"c_{dram.name}")
            nc.sync.dma_start(out=t[:], in_=dram[:, :])
            return t

        iota_t = _load_const(iota_d, [_P, _P], bf16)
        ident_b = _load_const(ident_b_d, [_P, _P], bf16)
        ident_f = cst.tile([_P, _P], f32, name="ident_f")
        make_identity(nc, ident_f[:])
        ws_t = [_load_const(wself[l], [_D, _D], bf16) for l in range(3)]
        wn_t = [_load_const(wneigh[l], [_D, _D], bf16) for l in range(3)]
        b_t = [_load_const(bias[l], [_D, 1]) for l in range(3)]
        g_t = [_load_const(gamma[l], [_D, 1]) for l in range(2)]
        be_t = [_load_const(beta[l], [_D, 1]) for l in range(2)]
        dwt_t = _load_const(disc_wt, [_D, _D])
        pw1_t = _load_const(pw1, [_D, _D], bf16)
        pw2_t = _load_const(pw2, [_D, _D], bf16)
        pw3_t = _load_const(pw3, [_D, 1], bf16)
        pb1_t = _load_const(pb1, [_D, 1])
        pb2_t = _load_const(pb2, [_D, 1])
        pb3_t = _load_const(pb3, [1, 1])
        ml2_t = _load_const(mask_l2_d, [_P, meta.n_win[2]])
        mpos_t = _load_const(mask_pos_d, [_P, 1])
        sig_t = _load_const(sigma_d, [_P, 1])

        zbufs = []
        for l in range(3):
            n_win = meta.n_win[l]
            q = meta.q[l]
            T = meta.T[l]
            h_src = xb if l == 0 else h_full[l - 1]
            zbuf = zp.tile([_P, n_win * _P], f32, name=f"zbuf{l}", tag=f"zbuf{l}")
            zbufs.append(zbuf)
            nc.gpsimd.memset(zbuf[:], 0.0)
            # whole-layer index metadata, one DMA each
            gi_l = mlp.tile([_P, T], i32, name=f"gi{l}", tag="meta_g")
            nc.sync.dma_start(out=gi_l[:], in_=gidx[l][:, :])
            dl_l = mlp.tile([_P, T], f32, name=f"dlm{l}", tag="meta_d")
            nc.sync.dma_start(out=dl_l[:], in_=dlocal[l][:, :])
            sc_l = mlp.tile([_P, T], f32, name=f"scm{l}", tag="meta_s")
            nc.sync.dma_start(out=sc_l[:], in_=escale[l][:, :])
            if l < 2:
                ssum = stp.tile([_P, n_win], f32, name=f"ssum{l}", tag=f"ssum{l}")
                ssq = stp.tile([_P, n_win], f32, name=f"ssq{l}", tag=f"ssq{l}")
            t_off = 0
            for w in range(n_win):
                nt = meta.tiles[l][w]
                valid = min(_P, q - w * _P)
                agg_self = psA.tile([_P, _P], f32, name="agg_self", tag="psA")
                have_edges = nt > 1
                if have_edges:
                    agg_nei = psB.tile([_P, _P], f32, name="agg_nei", tag="psB")
                t = 0
                while t < nt:
                    k = min(_GK, nt - t)
                    G = gp.tile([_P, k * _P], bf16, name="G", tag="G")
                    nc.gpsimd.indirect_dma_start(
                        out=G[:],
                        out_offset=None,
                        in_=h_src[:],
                        in_offset=bass.IndirectOffsetOnAxis(
                            ap=gi_l[:, t_off + t : t_off + t + k], axis=0
                        ),
                    )
                    for j in range(k):
                        tt = t + j
                        S = sp.tile([_P, _P], bf16, name="S", tag="S")
                        nc.vector.tensor_scalar(
                            out=S[:],
                            in0=iota_t[:],
                            scalar1=dl_l[:, t_off + tt : t_off + tt + 1],
                            scalar2=sc_l[:, t_off + tt : t_off + tt + 1],
                            op0=OP.is_equal,
                            op1=OP.mult,
                        )
                        Gj = G[:, j * _P : (j + 1) * _P]
                        if tt == 0:
                            nc.tensor.matmul(
                                out=agg_self[:], lhsT=Gj, rhs=S[:],
                                start=True, stop=True,
                            )
                        else:
                            nc.tensor.matmul(
                                out=agg_nei[:], lhsT=Gj, rhs=S[:],
                                start=(tt == 1), stop=(tt == nt - 1),
                            )
                    t += k
                a_self = wp.tile([_P, _P], bf16, name="a_self", tag="a_self")
                nc.vector.tensor_copy(out=a_self[:], in_=agg_self[:])
                if have_edges:
                    a_nei = wp.tile([_P, _P], bf16, name="a_nei", tag="a_nei")
                    nc.vector.tensor_copy(out=a_nei[:], in_=agg_nei[:])
                zt = psZ.tile([_P, _P], f32, name="zt", tag="psZ")
                nc.tensor.matmul(
                    out=zt[:], lhsT=ws_t[l][:], rhs=a_self[:],
                    start=True, stop=not have_edges,
                )
                if have_edges:
                    nc.tensor.matmul(
                        out=zt[:], lhsT=wn_t[l][:], rhs=a_nei[:],
                        start=False, stop=True,
                    )
                dstcol = zbufs[l][:, w * _P : w * _P + valid]
                if l < 2:
                    nc.scalar.activation(
                        out=dstcol,
                        in_=zt[:, :valid],
                        func=AF.Identity,
                        bias=b_t[l][:, :1],
                        accum_out=ssum[:, w : w + 1],
                    )
                    sqs = bp.tile([_P, _P], f32, name="sqs", tag="sqs")
                    nc.scalar.activation(
                        out=sqs[:, :valid],
                        in_=zt[:, :valid],
                        func=AF.Square,
                        bias=b_t[l][:, :1],
                        accum_out=ssq[:, w : w + 1],
                    )
                else:
                    nc.scalar.activation(
                        out=dstcol,
                        in_=zt[:, :valid],
                        func=AF.Identity,
                        bias=b_t[l][:, :1],
                    )
                t_off += nt

            if l < 2:
                ssum_tot = stp.tile([_P, 1], f32, name=f"ssumtot{l}", tag=f"sst{l}")
                nc.vector.tensor_reduce(
                    out=ssum_tot[:], in_=ssum[:], axis=AX.X, op=OP.add
                )
                ssq_tot = stp.tile([_P, 1], f32, name=f"ssqtot{l}", tag=f"ssqt{l}")
                nc.vector.tensor_reduce(
                    out=ssq_tot[:], in_=ssq[:], axis=AX.X, op=OP.add
                )
                nc.sync.dma_start(out=stats_in[l][:, 0:1], in_=ssum_tot[:])
                nc.sync.dma_start(out=stats_in[l][:, 1:2], in_=ssq_tot[:])
                nc.gpsimd.collective_compute(
                    "AllReduce",
                    OP.add,
                    replica_groups=rg_grp,
                    ins=[stats_in[l].ap()],
                    outs=[stats_out[l].ap()],
                )
                st = stp.tile([_P, 2], f32, name=f"st{l}", tag=f"stld{l}")
                nc.sync.dma_start(out=st[:], in_=stats_out[l][:, :])
                inv_n = 1.0 / float(_N[l + 1])
                mean = stp.tile([_P, 1], f32, name=f"mean{l}", tag=f"mean{l}")
                nc.scalar.activation(
                    out=mean[:], in_=st[:, 0:1], func=AF.Copy, scale=inv_n
                )
                ex2 = stp.tile([_P, 1], f32, name=f"ex2{l}", tag=f"ex2{l}")
                nc.scalar.activation(
                    out=ex2[:], in_=st[:, 1:2], func=AF.Copy, scale=inv_n
                )
                m2 = stp.tile([_P, 1], f32, name=f"m2{l}", tag=f"m2{l}")
                nc.vector.tensor_tensor(out=m2[:], in0=mean[:], in1=mean[:], op=OP.mult)
                var = stp.tile([_P, 1], f32, name=f"var{l}", tag=f"var{l}")
                nc.vector.tensor_tensor(
                    out=var[:], in0=ex2[:], in1=m2[:], op=OP.subtract
                )
                vpe = stp.tile([_P, 1], f32, name=f"vpe{l}", tag=f"vpe{l}")
                nc.vector.tensor_scalar_add(out=vpe[:], in0=var[:], scalar1=float(_EPS))
                rv = stp.tile([_P, 1], f32, name=f"rv{l}", tag=f"rv{l}")
                nc.vector.reciprocal(out=rv[:], in_=vpe[:])
                rstd = stp.tile([_P, 1], f32, name=f"rstd{l}", tag=f"rstd{l}")
                nc.scalar.activation(out=rstd[:], in_=rv[:], func=AF.Sqrt)
                scl = stp.tile([_P, 1], f32, name=f"scl{l}", tag=f"scl{l}")
                nc.vector.tensor_tensor(
                    out=scl[:], in0=g_t[l][:], in1=rstd[:], op=OP.mult
                )
                msc = stp.tile([_P, 1], f32, name=f"msc{l}", tag=f"msc{l}")
                nc.vector.tensor_tensor(
                    out=msc[:], in0=mean[:], in1=scl[:], op=OP.mult
                )
                shift = stp.tile([_P, 1], f32, name=f"shift{l}", tag=f"shift{l}")
                nc.vector.tensor_tensor(
                    out=shift[:], in0=be_t[l][:], in1=msc[:], op=OP.subtract
                )

            # pass B: (BN+ReLU) -> transpose -> node-major bf16 local shard
            for w in range(n_win):
                valid = min(_P, q - w * _P)
                src_cols = zbufs[l][:, w * _P : (w + 1) * _P]
                if l < 2:
                    tp = psT.tile([_P, _P], bf16, name="tp", tag="psT")
                    hb = bp.tile([_P, _P], bf16, name="hb", tag="hb")
                    nc.scalar.activation(
                        out=hb[:],
                        in_=src_cols,
                        func=AF.Relu,
                        scale=scl[:, :1],
                        bias=shift[:, :1],
                    )
                    nc.tensor.transpose(out=tp[:], in_=hb[:], identity=ident_b[:])
                else:
                    tp = psT.tile([_P, _P], f32, name="tp", tag="psT")
                    nc.tensor.transpose(out=tp[:], in_=src_cols, identity=ident_f[:])
                hn = bp.tile([_P, _P], bf16 if l < 2 else f32, name="hn", tag="hn")
                nc.vector.tensor_copy(out=hn[:valid, :], in_=tp[:valid, :])
                nc.sync.dma_start(
                    out=h_loc[l][w * _P : w * _P + valid, :], in_=hn[:valid, :]
                )
            nc.gpsimd.collective_compute(
                "AllGather",
                OP.bypass,
                replica_groups=rg_grp,
                ins=[h_loc[l].ap()],
                outs=[h_full[l].ap()],
            )

        # ---- final stage ----
        spart = fp.tile([_P, 1], f32, name="spart", tag="spart")
        nc.vector.tensor_reduce(out=spart[:], in_=zbufs[2][:], axis=AX.X, op=OP.add)
        nc.vector.tensor_scalar_mul(out=spart[:], in0=spart[:], scalar1=mpos_t[:, :1])
        nc.sync.dma_start(out=sumr_in[:, :], in_=spart[:])
        nc.gpsimd.collective_compute(
            "AllReduce",
            OP.add,
            replica_groups=rg_all,
            ins=[sumr_in.ap()],
            outs=[sumr_out.ap()],
        )
        ssumv = fp.tile([_P, 1], f32, name="ssumv", tag="ssumv")
        nc.sync.dma_start(out=ssumv[:], in_=sumr_out[:, :])
        summary = fp.tile([_P, 1], f32, name="summary", tag="summary")
        nc.scalar.activation(
            out=summary[:], in_=ssumv[:], func=AF.Sigmoid, scale=1.0 / float(_N[3])
        )
        ws_ps = psA.tile([_P, 1], f32, name="ws_ps", tag="psA")
        nc.tensor.matmul(
            out=ws_ps[:], lhsT=dwt_t[:], rhs=summary[:], start=True, stop=True
        )
        wsv = fp.tile([_P, 1], f32, name="wsv", tag="wsv")
        nc.vector.tensor_copy(out=wsv[:], in_=ws_ps[:])

        loss_ps = psT.tile([1, 1], f32, name="loss_ps", tag="psT")
        for w in range(meta.n_win[2]):
            sc_ps = psB.tile([_P, 1], f32, name="sc_ps", tag="psB")
            nc.tensor.matmul(
                out=sc_ps[:],
                lhsT=zbufs[2][:, w * _P : (w + 1) * _P],
                rhs=wsv[:],
                start=True,
                stop=True,
            )
            spu = fp.tile([_P, 1], f32, name="spu", tag="spu")
            nc.vector.tensor_scalar(
                out=spu[:],
                in0=sc_ps[:],
                scalar1=sig_t[:, :1],
                scalar2=30.0,
                op0=OP.mult,
                op1=OP.min,
            )
            spe = fp.tile([_P, 1], f32, name="spe", tag="spe")
            nc.scalar.activation(out=spe[:], in_=spu[:], func=AF.Exp)
            sp1 = fp.tile([_P, 1], f32, name="sp1", tag="sp1")
            nc.vector.tensor_scalar_add(out=sp1[:], in0=spe[:], scalar1=1.0)
            spl = fp.tile([_P, 1], f32, name="spl", tag="spl")
            nc.scalar.activation(out=spl[:], in_=sp1[:], func=AF.Ln)
            nc.tensor.matmul(
                out=loss_ps[:],
                lhsT=spl[:],
                rhs=ml2_t[:, w : w + 1],
                start=(w == 0),
                stop=(w == meta.n_win[2] - 1),
            )
        lsum = fp.tile([1, 1], f32, name="lsum", tag="lsum")
        nc.scalar.activation(
            out=lsum[:], in_=loss_ps[:], func=AF.Copy, scale=1.0 / float(_N[3])
        )
        nc.sync.dma_start(out=loss_in[:, :], in_=lsum[:])
        nc.gpsimd.collective_compute(
            "AllReduce",
            OP.add,
            replica_groups=rg_all,
            ins=[loss_in.ap()],
            outs=[loss_red.ap()],
        )
        lfin = fp.tile([1, 1], f32, name="lfin", tag="lfin")
        nc.sync.dma_start(out=lfin[:], in_=loss_red[:, :])
        nc.sync.dma_start(out=loss_out[:, :], in_=lfin[:])

        # predictor MLP over pair tiles (positive_full = h_full[2])
        pa_t = cst.tile([_P, meta.n_pair_tiles], i32, name="pa_t")
        nc.sync.dma_start(out=pa_t[:], in_=pair_a_d[:, :])
        pb_t = cst.tile([_P, meta.n_pair_tiles], i32, name="pb_t")
        nc.sync.dma_start(out=pb_t[:], in_=pair_b_d[:, :])
        for t in range(meta.n_pair_tiles):
            A = gp.tile([_P, _P], f32, name="A", tag="G")
            nc.gpsimd.indirect_dma_start(
                out=A[:],
                out_offset=None,
                in_=h_full[2][:],
                in_offset=bass.IndirectOffsetOnAxis(ap=pa_t[:, t : t + 1], axis=0),
            )
            B = sp.tile([_P, _P], f32, name="B", tag="S")
            nc.gpsimd.indirect_dma_start(
                out=B[:],
                out_offset=None,
                in_=h_full[2][:],
                in_offset=bass.IndirectOffsetOnAxis(ap=pb_t[:, t : t + 1], axis=0),
            )
            Z = bp.tile([_P, _P], f32, name="Z", tag="hb")
            nc.vector.tensor_tensor(out=Z[:], in0=A[:], in1=B[:], op=OP.mult)
            ztp = psT.tile([_P, _P], f32, name="ztp", tag="psT")
            nc.tensor.transpose(out=ztp[:], in_=Z[:], identity=ident_f[:])
            zin = bp.tile([_P, _P], bf16, name="zin", tag="hn")
            nc.vector.tensor_copy(out=zin[:], in_=ztp[:])
            z1p = psA.tile([_P, _P], f32, name="z1p", tag="psA")
            nc.tensor.matmul(
                out=z1p[:], lhsT=pw1_t[:], rhs=zin[:], start=True, stop=True
            )
            z1 = wp.tile([_P, _P], bf16, name="z1", tag="a_self")
            nc.scalar.activation(out=z1[:], in_=z1p[:], func=AF.Relu, bias=pb1_t[:, :1])
            z2p = psB.tile([_P, _P], f32, name="z2p", tag="psB")
            nc.tensor.matmul(
                out=z2p[:], lhsT=pw2_t[:], rhs=z1[:], start=True, stop=True
            )
            z2 = wp.tile([_P, _P], bf16, name="z2", tag="a_nei")
            nc.scalar.activation(out=z2[:], in_=z2p[:], func=AF.Relu, bias=pb2_t[:, :1])
            hp_ps = psZ.tile([1, _P], f32, name="hp_ps", tag="psZ")
            nc.tensor.matmul(
                out=hp_ps[:], lhsT=pw3_t[:], rhs=z2[:], start=True, stop=True
            )
            ho = fp.tile([1, _P], f32, name="ho", tag="ho")
            nc.scalar.activation(
                out=ho[:], in_=hp_ps[:], func=AF.Identity, bias=pb3_t[:1, :1]
            )
            nc.sync.dma_start(out=pair_out[t : t + 1, :], in_=ho[:])

    nc.compile()
    return nc


_CACHE = {}


def _get_program(meta):
    k = meta.key()
    if k not in _CACHE:
        _CACHE[k] = _build(meta)
    return _CACHE[k]


LAST_RESULTS = None


def _install_ntff_hook():
    """Provide antenv.axon_hooks (missing in this image) so
    run_bass_kernel_spmd(trace=True) can capture NTFF profiles."""
    import types
    import ctypes
    import contextlib

    try:
        from antenv.axon_hooks import get_axon_ntff_profile_hook  # noqa: F401

        return True
    except ImportError:
        pass
    so_path = "/opt/axon/libaxon_pjrt.so"
    if not os.path.exists(so_path):
        return False
    lib = ctypes.CDLL(so_path)
    if not hasattr(lib, "axon_start_nrt_profile"):
        return False
    lib.axon_start_nrt_profile.argtypes = [
        ctypes.POINTER(ctypes.c_int64),
        ctypes.c_size_t,
    ]
    lib.axon_start_nrt_profile.restype = ctypes.c_int64
    lib.axon_stop_nrt_profile.argtypes = [ctypes.c_char_p]
    lib.axon_stop_nrt_profile.restype = ctypes.c_int64

    @contextlib.contextmanager
    def _hook(output_dir, device_ids):
        import jax

        jax.devices()
        if device_ids:
            ids = (ctypes.c_int64 * len(device_ids))(*device_ids)
            rc = lib.axon_start_nrt_profile(ids, len(device_ids))
        else:
            rc = lib.axon_start_nrt_profile(None, 0)
        if rc != 0:
            raise RuntimeError(f"axon_start_nrt_profile rc={rc}")
        try:
            yield
        finally:
            n = lib.axon_stop_nrt_profile(str(output_dir).encode())
            print(f"ntff profile: {n} file(s) -> {output_dir}", file=sys.stderr)

    mod = types.ModuleType("antenv.axon_hooks")
    mod.get_axon_ntff_profile_hook = lambda: _hook
    mod.set_axon_ntff_profile_hook = lambda h: None
    import antenv

    antenv.axon_hooks = mod
    sys.modules["antenv.axon_hooks"] = mod
    return True


def kernel(**inputs):
    global LAST_RESULTS
    _ensure_paths()
    from concourse import bass_utils

    in_maps, meta = _prepare(inputs)
    nc = _get_program(meta)
    trace = bool(os.environ.get("KERNEL_TRACE"))
    if trace:
        trace = _install_ntff_hook()
    res = bass_utils.run_bass_kernel_spmd(
        nc, in_maps, core_ids=list(range(_NCORES)), trace=trace
    )
    LAST_RESULTS = res

    n_pairs = 2 * _E_PAIR
    flat = np.concatenate(
        [
            res.results[c]["pair_out"].reshape(-1)[: meta.pair_per_core]
            for c in range(_NGRP)
        ]
    )[:n_pairs]
    h_pos = flat[:_E_PAIR].reshape(_E_PAIR, 1).astype(np.float32)
    h_neg = flat[_E_PAIR:].reshape(_E_PAIR, 1).astype(np.float32)
    loss = np.float32(res.results[0]["loss_out"].reshape(-1)[0])
    return h_pos, h_neg, loss
